# revision 38
# baseline (speedup 1.0000x reference)
"""Causal self-attention Trainium2 Bass kernel (v2).

Problem: B=4, S=2048, C=1024, H=16 heads, D=64 head_dim.
  qkv = x @ qkv_w.T + qkv_b ; per-head causal softmax attention ; out = attn @ proj_w.T + proj_b

Sharding (8 cores): core = 2*b + hg  (data parallel over batch b=0..3,
tensor parallel over 2 head-groups of 8 heads).  Each core computes
q/k/v for its 8 heads over the full sequence, does causal attention
locally, and computes a partial output projection (contraction over its
512 channels).  Host sums the two partials per batch.

Changes over the original baseline (~332us -> ~297us):
  - paired-parity score psum: one [128, 2(head), 512] tile per k-block;
    both heads' score matmuls (K=64, disjoint PE row halves) depend on
    the same tile allocation, so they issue back-to-back and stream
    concurrently on the PE's row-group subarrays; one exp per k-block
    covers both heads.
  - narrowing-matmul causal band: diagonal-band k-block kb (relative
    index i = kb - 4*q0) only computes score columns q >= 128*i
    (N = 512-128*i), the optimal 136-miniblock causal cover (vs 160).
    The mask multiply shrinks to the single leading 128x128 mini,
    applied per-head so head-0's AV matmul starts earlier.
  - normalize path: copies emitted for both heads first (frees PSUM
    fast), then reciprocal on a [128,4] spread, then the DRAM-bounce
    broadcast; all kept on DVE/SP (GPSIMD showed an intermittent
    hardware-level sync corruption despite a clean CoreSim race check).
  - host-side p-major input layouts (long per-partition DMA runs), DMA
    waves issued from both SP and ACT matched to first consumption,
    x chunk 0 split in halves for an early first matmul, and late
    input waves emitted inside the pair-0 loop so normalize DMAs don't
    queue behind bulk input on the SP sequencer.
  - stage-1 loops s-chunk-outer for pair 0 with attention(q0) emitted
    as soon as its s-chunk is done, so the exp stream starts early;
    pairs 1-3's stage-1 groups fill later ACT-bound attention
    stretches via the tile scheduler.
  - output tiles written in 2-4 chunks to cut the final DMA drain tail.
"""

import numpy as np
import ml_dtypes

import concourse.bacc as bacc
import concourse.bass as bass
import concourse.mybir as mybir
import concourse.tile as tile
from concourse.bass_utils import run_bass_kernel_spmd

BF = ml_dtypes.bfloat16
F32 = mybir.dt.float32
BF16 = mybir.dt.bfloat16
EXP = mybir.ActivationFunctionType.Exp

B, S, C = 4, 2048, 1024
H, D = 16, 64
P = 128
NQ = 512            # q-chunk
NSQ = S // NQ       # 4 q-chunks
NKB = S // P        # 16 k-blocks
CO = C // P         # 8 contraction tiles for stage 1
CPH = 512           # channels per head-group (8 heads * 64)

LAST_RESULTS = None
_NC_CACHE = []


def _ensure_axon_hooks():
    """Provide antenv.axon_hooks (NTFF profile hook) when the image lacks it."""
    import sys
    import types
    import contextlib
    import ctypes
    import os

    from concourse import bass_utils as _bu
    _bu.upload_artifacts = lambda tmpdir: str(tmpdir)

    try:
        import antenv.axon_hooks  # noqa: F401
        return
    except ImportError:
        pass

    state = {}

    def set_axon_ntff_profile_hook(hook):
        state["hook"] = hook

    def get_axon_ntff_profile_hook():
        if "hook" in state:
            return state["hook"]
        so = "/opt/axon/libaxon_pjrt.so"
        if not os.path.exists(so):
            return None
        lib = ctypes.CDLL(so)
        if not hasattr(lib, "axon_start_nrt_profile"):
            return None
        lib.axon_start_nrt_profile.argtypes = [
            ctypes.POINTER(ctypes.c_int64), ctypes.c_size_t]
        lib.axon_start_nrt_profile.restype = ctypes.c_int64
        lib.axon_stop_nrt_profile.argtypes = [ctypes.c_char_p]
        lib.axon_stop_nrt_profile.restype = ctypes.c_int64

        @contextlib.contextmanager
        def _hook(output_dir, device_ids):
            import jax
            jax.devices()
            if device_ids:
                ids = (ctypes.c_int64 * len(device_ids))(*device_ids)
                rc = lib.axon_start_nrt_profile(ids, len(device_ids))
            else:
                rc = lib.axon_start_nrt_profile(None, 0)
            if rc != 0:
                raise RuntimeError(f"axon_start_nrt_profile rc={rc}")
            try:
                yield
            finally:
                n = lib.axon_stop_nrt_profile(str(output_dir).encode())
                print(f"ntff profile: {n} file(s) written to {output_dir}")

        state["hook"] = _hook
        return _hook

    import antenv
    mod = types.ModuleType("antenv.axon_hooks")
    mod.set_axon_ntff_profile_hook = set_axon_ntff_profile_hook
    mod.get_axon_ntff_profile_hook = get_axon_ntff_profile_hook
    sys.modules["antenv.axon_hooks"] = mod
    antenv.axon_hooks = mod


def _qk_col(co):
    """Column range start in wT for stage-1 tile co (pair-interleaved q/k)."""
    return 256 * (co % 4) + (0 if co < 4 else P)


def _build_program():
    nc = bacc.Bacc("TRN2", target_bir_lowering=False, debug=False)

    # p-major host layouts: index [partition, o, cols]
    xh = nc.dram_tensor("xh", [P, CO, S], BF16, kind="ExternalInput")
    wh = nc.dram_tensor("wh", [P, CO, 3 * CPH], BF16, kind="ExternalInput")
    qkb = nc.dram_tensor("qkb", [P, 8], F32, kind="ExternalInput")
    bvb = nc.dram_tensor("bvb", [P, CPH], F32, kind="ExternalInput")
    pwh = nc.dram_tensor("pwh", [P, CPH // P, C], BF16, kind="ExternalInput")
    pbb = nc.dram_tensor("pbb", [P, C], F32, kind="ExternalInput")
    dmask = nc.dram_tensor("dmask", [P, 2, P], BF16, kind="ExternalInput")
    out = nc.dram_tensor("out", [S, C], F32, kind="ExternalOutput")

    with tile.TileContext(nc) as tc:
        with (
            tc.tile_pool(name="const", bufs=1) as const,
            tc.tile_pool(name="work", bufs=4) as work,
            tc.tile_pool(name="psg", bufs=2, space="PSUM") as psum_gen,
            tc.tile_pool(name="pss", bufs=2, space="PSUM") as psum_sc,
            tc.tile_pool(name="psa", bufs=2, space="PSUM") as psum_av,
            tc.tile_pool(name="dram", bufs=4, space="DRAM") as dram,
        ):
            # ---- persistent SBUF ----
            xT_sb = const.tile([P, CO, S], BF16, tag="xT", name="xT_sb")
            wT_sb = const.tile([P, CO, 3 * CPH], BF16, tag="wT", name="wT_sb")
            qkb_sb = const.tile([P, 8], F32, tag="qkb", name="qkb_sb")
            bvb_sb = const.tile([P, CPH], F32, tag="bvb", name="bvb_sb")
            pwT_sb = const.tile([P, CPH // P, C], BF16, tag="pwT", name="pwT_sb")
            pbb_sb = const.tile([P, C], F32, tag="pbb", name="pbb_sb")
            dm_sb = const.tile([P, 2, P], BF16, tag="dmask", name="dm_sb")

            qT_sb = [const.tile([P, S], BF16, tag=f"qT{p}", name=f"qT_sb{p}") for p in range(4)]
            kT_sb = [const.tile([P, S], BF16, tag=f"kT{p}", name=f"kT_sb{p}") for p in range(4)]
            # v: [s-part, kb, pair*2+par, d+ones]
            v_all = const.tile([P, NKB, 8, D + 1], BF16, tag="v", name="v_all")
            aT_sb = [const.tile([P, S], BF16, tag=f"aT{p}", name=f"aT_sb{p}") for p in range(4)]

            nc.vector.memset(v_all[:, :, :, D:D + 1], 1.0)

            # PE warm-up: ~35 junk matmuls with no DMA deps run during the
            # initial input-DMA wait, so the HAM clock gate is at 2.4 GHz
            # when the first real matmuls arrive (saves the 1.2 GHz cold
            # stretch).  Results are never read.
            warm_sb = const.tile([P, P], BF16, tag="warm", name="warm_sb")
            nc.vector.memset(warm_sb, 0.0)
            ps_warm = psum_gen.tile([P, NQ], F32, tag="gen", name="ps_warm")
            for i in range(35):
                nc.tensor.matmul(ps_warm[:, 0:P], lhsT=warm_sb, rhs=warm_sb,
                                 start=True, stop=True)

            # ---- input DMA waves (wave A + early wave B) ----
            # Only SP (sync) and Activation (scalar) can issue DMAs, at
            # ~0.6us of sequencer time per dma_start.  ACT must be free for
            # exps from ~14us on, so it only gets the early weight waves;
            # the late bulk is emitted interleaved into the pair-0 loop so
            # normalize DMAs don't queue behind it on SP.
            for o in range(CO):     # pair-0 qk weight cols + x chunk 0 halves
                nc.scalar.dma_start(out=wT_sb[:, o, 0:256], in_=wh[:, o, 0:256])
                nc.sync.dma_start(out=xT_sb[:, o, 0:256], in_=xh[:, o, 0:256])
                nc.scalar.dma_start(out=xT_sb[:, o, 256:512], in_=xh[:, o, 256:512])
            nc.sync.dma_start(out=qkb_sb, in_=qkb[:, :])
            nc.sync.dma_start(out=dm_sb, in_=dmask[:, :, :])
            nc.sync.dma_start(out=bvb_sb, in_=bvb[:, :])
            for o in range(CO):     # v cols (ACT), x chunk 1 (SP)
                nc.scalar.dma_start(out=wT_sb[:, o, 1024:1536], in_=wh[:, o, 1024:1536])
                nc.sync.dma_start(out=xT_sb[:, o, NQ:2 * NQ], in_=xh[:, o, NQ:2 * NQ])
            for o in range(CO):     # pair-1 qk cols (ACT), pair-2 (SP)
                nc.scalar.dma_start(out=wT_sb[:, o, 256:512], in_=wh[:, o, 256:512])
                nc.sync.dma_start(out=wT_sb[:, o, 512:768], in_=wh[:, o, 512:768])
            for o in range(CO):     # pair-3 (SP)
                nc.sync.dma_start(out=wT_sb[:, o, 768:1024], in_=wh[:, o, 768:1024])

            def late_input_wave(sq):
                # emitted inside the pair-0 loop: x chunks 2-3, proj
                # weights/bias, in consumption order
                if sq == 0:
                    for o in range(CO):
                        nc.sync.dma_start(out=xT_sb[:, o, 2 * NQ:3 * NQ],
                                          in_=xh[:, o, 2 * NQ:3 * NQ])
                elif sq == 1:
                    for o in range(CO):
                        nc.sync.dma_start(out=xT_sb[:, o, 3 * NQ:4 * NQ],
                                          in_=xh[:, o, 3 * NQ:4 * NQ])
                elif sq == 2:
                    for o4 in range(CPH // P):   # proj weights (needed at stage 3)
                        nc.sync.dma_start(out=pwT_sb[:, o4, 0:NQ], in_=pwh[:, o4, 0:NQ])
                        nc.sync.dma_start(out=pwT_sb[:, o4, NQ:C], in_=pwh[:, o4, NQ:C])
                    for c4 in range(4):
                        nc.sync.dma_start(out=pbb_sb[:, c4 * 256:(c4 + 1) * 256],
                                          in_=pbb[:, c4 * 256:(c4 + 1) * 256])

            def qk_group(co, sq):
                """One [128, 512] tile of qT/kT for pair co%4."""
                dst = qT_sb[co] if co < 4 else kT_sb[co - 4]
                wc = _qk_col(co)
                ps = psum_gen.tile([P, NQ], F32, tag="gen", name=f"ps_qk_{co}_{sq}")
                for kc in range(CO):
                    nc.tensor.matmul(
                        ps,
                        lhsT=wT_sb[:, kc, wc:wc + P],
                        rhs=xT_sb[:, kc, sq * NQ:(sq + 1) * NQ],
                        start=(kc == 0), stop=(kc == CO - 1),
                    )
                nc.vector.tensor_scalar_add(
                    out=dst[:, sq * NQ:(sq + 1) * NQ], in0=ps,
                    scalar1=qkb_sb[:, co:co + 1],
                )

            def v_group(st):
                ps = psum_gen.tile([P, CPH], F32, tag="gen", name=f"ps_v_{st}")
                for kc in range(CO):
                    nc.tensor.matmul(
                        ps,
                        lhsT=xT_sb[:, kc, st * P:(st + 1) * P],
                        rhs=wT_sb[:, kc, 2 * CPH:3 * CPH],
                        start=(kc == 0), stop=(kc == CO - 1),
                    )
                nc.vector.tensor_add(
                    out=v_all[:, st, :, 0:D],
                    in0=ps.rearrange("q (g d) -> q g d", g=8),
                    in1=bvb_sb.rearrange("q (g d) -> q g d", g=8),
                )

            # Deferred normalize tail: the reciprocal (waits on the r4 DMA)
            # and the final multiply (waits on the bcs DMA) would otherwise
            # stall the in-order DVE stream and block the queued stage-1
            # bias-adds -> psum recycling -> PE.  Recips run one attention
            # chunk later (r4 has landed), muls two chunks later (bcs has
            # landed).  avs/r4/bcs/rdr bufs=6 cover the three live chunks.
            pending_recips = []
            pending_muls = []

            def flush_norm_tail(keep=1):
                while pending_recips:
                    pending_recips.pop(0)()
                while len(pending_muls) > keep:
                    pending_muls.pop(0)()

            def attention_q0(pr, q0):
                """Causal attention for pair pr, q-chunk q0 (both heads)."""
                qlo = q0 * NQ
                nkb = 4 * (q0 + 1)
                flush_norm_tail(keep=1)
                avs_ps = [psum_av.tile([D + 1, NQ], F32, tag="av",
                                       name=f"av_{pr}_{q0}_{par}") for par in range(2)]
                for kb in range(nkb):
                    qoff = P * max(0, kb - 4 * q0)
                    sc = psum_sc.tile([P, 2, NQ], F32, tag="sc",
                                      name=f"sc_{pr}_{q0}_{kb}")
                    for par in range(2):
                        base = par * D
                        nc.tensor.matmul(
                            sc[:, par, qoff:NQ],
                            lhsT=kT_sb[pr][base:base + D, kb * P:(kb + 1) * P],
                            rhs=qT_sb[pr][base:base + D, qlo + qoff:qlo + NQ],
                            start=True, stop=True,
                        )
                    pt = work.tile([P, 2, NQ], BF16, tag="pt", bufs=6,
                                   name=f"pt_{pr}_{q0}_{kb}")
                    nc.scalar.activation(out=pt[:, :, qoff:NQ], in_=sc[:, :, qoff:NQ],
                                         func=EXP, scale=0.125)
                    for par in range(2):
                        if kb >= 4 * q0:    # diagonal mini needs the causal mask
                            # per-par so AV-par0 starts before par1's mask
                            nc.vector.tensor_mul(
                                out=pt[:, par, qoff:qoff + P],
                                in0=pt[:, par, qoff:qoff + P], in1=dm_sb[:, par, :])
                        nc.tensor.matmul(
                            avs_ps[par][:, qoff:NQ],
                            lhsT=v_all[:, kb, 2 * pr + par, :],
                            rhs=pt[:, par, qoff:NQ],
                            start=(kb == 0), stop=(kb == nkb - 1),
                            skip_group_check=True,
                        )
                # normalize: av[0:64]/av[64] -> aT (bf16).  copies first (free
                # the PSUM banks fast), then the reciprocal on a [128,4]
                # spread, then the DRAM-bounce broadcast back to a [64,512]
                # divisor; emitted par-interleaved so the DMA latencies of the
                # two parities overlap.
                avs = [work.tile([D + 1, NQ], F32, tag="avs", bufs=6,
                                 name=f"avs_{pr}_{q0}_{par}") for par in range(2)]
                r4 = [work.tile([P, 4], F32, tag="r4", bufs=6,
                                name=f"r4_{pr}_{q0}_{par}") for par in range(2)]
                rdr = [dram.tile([NQ], F32, tag="rdr", bufs=6,
                                 name=f"rdr_{pr}_{q0}_{par}") for par in range(2)]
                bcs = [work.tile([D, NQ], F32, tag="bcs", bufs=6,
                                 name=f"bcs_{pr}_{q0}_{par}") for par in range(2)]
                for par in range(2):
                    nc.vector.tensor_copy(out=avs[par], in_=avs_ps[par])
                for par in range(2):
                    nc.sync.dma_start(out=r4[par], in_=avs[par][D:D + 1, :])

                def recips(r4=r4, rdr=rdr, bcs=bcs):
                    for par in range(2):
                        nc.vector.reciprocal(out=r4[par], in_=r4[par])
                        nc.sync.dma_start(out=rdr[par][:], in_=r4[par])
                    for par in range(2):
                        rdr_bcast = bass.AP(
                            tensor=rdr[par].tensor, offset=rdr[par].offset,
                            ap=[[0, D], rdr[par].ap[0]],
                        )
                        nc.sync.dma_start(out=bcs[par], in_=rdr_bcast)
                pending_recips.append(recips)

                def muls(pr=pr, qlo=qlo, avs=avs, bcs=bcs):
                    for par in range(2):
                        nc.vector.tensor_mul(
                            out=aT_sb[pr][par * D:(par + 1) * D, qlo:qlo + NQ],
                            in0=avs[par][0:D, :], in1=bcs[par],
                        )
                pending_muls.append(muls)

            def stage3_wave(q0):
                for st in range(4 * q0, 4 * q0 + 4):
                    for c2 in range(2):
                        ps = psum_gen.tile([P, NQ], F32, tag="gen", name=f"ps_o_{st}_{c2}")
                        for o in range(4):
                            nc.tensor.matmul(
                                ps,
                                lhsT=aT_sb[o][:, st * P:(st + 1) * P],
                                rhs=pwT_sb[:, o, c2 * NQ:(c2 + 1) * NQ],
                                start=(o == 0), stop=(o == 3),
                            )
                        ot = work.tile([P, NQ], F32, tag="out", name=f"ot_{st}_{c2}")
                        nc.vector.tensor_add(out=ot, in0=ps, in1=pbb_sb[:, c2 * NQ:(c2 + 1) * NQ])
                        nch = 4 if st >= NKB - 2 else 2
                        w = NQ // nch
                        for ch in range(nch):
                            eng = nc.sync if (ch % 2 == 0) else nc.scalar
                            eng.dma_start(
                                out=out[st * P:(st + 1) * P,
                                        c2 * NQ + ch * w:c2 * NQ + (ch + 1) * w],
                                in_=ot[:, ch * w:(ch + 1) * w],
                            )

            # ---- emission ----
            # pair 0: stream stage 1 s-chunk-major and start attention on
            # each q-chunk as soon as its q/k/v tiles exist; pairs 1-3 after,
            # with their stage-1 groups filling earlier ACT-bound attention
            # stretches via the tile scheduler; stage 3 last.
            for sq in range(NSQ):
                qk_group(0, sq)
                qk_group(4, sq)
                for st in range(4 * sq, 4 * sq + 4):
                    v_group(st)
                late_input_wave(sq)
                attention_q0(0, sq)
            for pr in range(1, 4):
                for sq in range(NSQ):
                    qk_group(pr, sq)
                    qk_group(4 + pr, sq)
                for q0 in range(NSQ):
                    attention_q0(pr, q0)
            flush_norm_tail(keep=0)
            for q0 in range(NSQ):
                stage3_wave(q0)

    nc.compile()
    return nc


def _get_nc():
    if not _NC_CACHE:
        _NC_CACHE.append(_build_program())
    return _NC_CACHE[0]


def _make_in_maps(x, qkv_w, qkv_b, proj_w, proj_b):
    x = np.asarray(x, np.float32)
    qkv_w = np.asarray(qkv_w, np.float32)
    qkv_b = np.asarray(qkv_b, np.float32)
    proj_w = np.asarray(proj_w, np.float32)
    proj_b = np.asarray(proj_b, np.float32)

    # causal mask for the leading 128x128 mini of a diagonal-band k-block,
    # duplicated for both heads of a pair
    kk = np.arange(P)[:, None]
    qq = np.arange(P)[None, :]
    tril = (kk <= qq).astype(BF)
    dmask = np.ascontiguousarray(np.broadcast_to(tril[:, None, :], (P, 2, P)))

    in_maps = []
    for core in range(8):
        b, hg = core // 2, core % 2
        rows = slice(hg * CPH, (hg + 1) * CPH)
        wq = qkv_w[0 * C:1 * C][rows]     # [512, 1024]
        wk = qkv_w[1 * C:2 * C][rows]
        wv = qkv_w[2 * C:3 * C][rows]
        # col order: q0|k0|q1|k1|q2|k2|q3|k3|v  (128-wide q/k blocks per pair)
        blocks = []
        for pr in range(4):
            blocks.append(wq[pr * P:(pr + 1) * P])
            blocks.append(wk[pr * P:(pr + 1) * P])
        blocks.append(wv)
        w_shard = np.concatenate(blocks, axis=0)          # [1536, 1024]
        wT = np.ascontiguousarray(w_shard.T)              # [1024, 1536]
        wh = np.ascontiguousarray(
            wT.reshape(CO, P, 3 * CPH).transpose(1, 0, 2)).astype(BF)

        xT = np.ascontiguousarray(x[b].T)                 # [1024, 2048]
        xh = np.ascontiguousarray(
            xT.reshape(CO, P, S).transpose(1, 0, 2)).astype(BF)

        bq = qkv_b[0 * C:][rows]
        bk = qkv_b[1 * C:][rows]
        bv = qkv_b[2 * C:][rows]

        pwT = np.ascontiguousarray(proj_w[:, rows].T)     # [512, 1024]
        pwh = np.ascontiguousarray(
            pwT.reshape(CPH // P, P, C).transpose(1, 0, 2)).astype(BF)

        in_maps.append({
            "xh": xh,
            "wh": wh,
            "qkb": np.ascontiguousarray(
                np.concatenate([bq, bk]).reshape(8, P).T).astype(np.float32),
            "bvb": np.ascontiguousarray(np.tile(bv[None, :], (P, 1))).astype(np.float32),
            "pwh": pwh,
            "pbb": (np.tile(proj_b[None, :], (P, 1)).astype(np.float32)
                    if hg == 0 else np.zeros((P, C), np.float32)),
            "dmask": dmask,
        })
    return in_maps


def kernel(x, qkv_w, qkv_b, proj_w, proj_b, _trace=False):
    global LAST_RESULTS
    _ensure_axon_hooks()
    in_maps = _make_in_maps(x, qkv_w, qkv_b, proj_w, proj_b)
    nc = _get_nc()
    res = run_bass_kernel_spmd(nc, in_maps, core_ids=list(range(8)), trace=_trace)
    LAST_RESULTS = res
    out = np.empty((B, S, C), np.float32)
    for b in range(B):
        out[b] = res.results[2 * b]["out"] + res.results[2 * b + 1]["out"]
    return out
